# revision 20
# baseline (speedup 1.0000x reference)
"""ListFoldLoss Trainium2 kernel (8-core SPMD, Bass/Tile).

Math: the reference builds D[u,v] = exp(f_u - f_v) (rank-1: exp(f) x exp(-f))
and takes nested-window sums of it.  Every window sum factorizes:

    S(i) = A(i) * B(i),   A(i) = sum_{rank in [i, N-i)} exp(pred),
                          B(i) = sum_{rank in [i, N-i)} exp(-pred)

so the NxN matrix never needs to exist.  With r_u = rank of element u when
sorting by target descending and m_u = min(r_u, N-1-r_u):

    A(i)  = sum_u exp(pred_u)  * [m_u >= i]
    B(i)  = sum_u exp(-pred_u) * [m_u >= i]
    num   = sum_u pred_u * (2*[r_u < N/2] - 1)        (= sum_i log num_i)
    loss  = sum_i log(A(i)*B(i) - (N-2i)) - num

Device plan (per core c of 8):
  phase 1: ranks r_u for u in the core's 1024-slice, via N comparisons per u:
           DVE fused tensor_scalar(is_gt)+accum_out for 5 of 8 u-subtiles and
           ACT fused sign(t_j - t_u)+accum_out for 3 (runs concurrently).
  phase 1.5: m = min(r, N-1-r), local num partial; AllGather m (4KB/core).
  phase 2: W[u,i] = [i <= m_u] (DVE tensor_scalar is_le, 2x mode) and
           PSUM accumulation of (a,b)^T @ W over 64 u-subtiles (PE matmuls).
  phase 3: denom = A*B - win, ln via ACT (+fused sum), minus num partial.
Host: sums the 8 per-core partial losses (pure unshard of a sum-sharded scalar).

`reps` replicates the whole body serially inside one NEFF — used only for
slope-based wall-clock timing (NTFF profiling is unavailable here).
"""

import numpy as np

import concourse.bacc as bacc
import concourse.bass as bass
import concourse.mybir as mybir
import concourse.tile as tile

N = 8192
NCORE = 8
P = 128
US = N // NCORE          # 1024 u's per core
UT = US // P             # 8 u-subtiles per core
NPAIR = N // 2           # 4096 loss terms
IS = NPAIR // NCORE      # 512 i's per core
NCHUNK = 2               # j-dim chunks for DMA/compute pipelining
CH = N // NCHUNK
NUTILE = N // P          # 64 u-subtiles globally (phase 2)
DVE_K = (0, 1, 2, 3, 4)  # u-subtiles ranked on the vector engine
ACT_K = (5, 6, 7)        # u-subtiles ranked on the scalar engine (sign trick)
# phase-2 mask tiles handled by ACT (sign trick, mod-8 position), rest on DVE
ACT2_MOD = (5, 6, 7)

F32 = mybir.dt.float32
AF = mybir.ActivationFunctionType
OP = mybir.AluOpType

# "f32" = correct but 4 cyc/row on PE; "f32r" = full rate, reduced precision;
# "bf16" = full rate, ab operands rounded to bf16.
MATMUL_MODE = "f32r"


def build_module(
    debug: bool = False,
    reps: int = 1,
    collective: bool = True,
    n1d: int = len(DVE_K),
    act2_mod: tuple = ACT2_MOD,
    nchunk: int = NCHUNK,
    work_bufs: int = 2,
    w_bufs: int = 4,
):
    dve_k = tuple(range(n1d))
    act_k = tuple(range(n1d, UT))
    # progressive chunk sizes: small first chunk so compares start early,
    # large later chunks to amortize per-instruction overhead
    if nchunk == 2:
        bounds = (0, 1024, 4096, N)
    else:
        bounds = tuple(q * (N // nchunk) for q in range(nchunk)) + (N,)
    nch = len(bounds) - 1
    nc = bacc.Bacc(
        "TRN2",
        target_bir_lowering=False,
        debug=False,
        enable_asserts=False,
        num_devices=NCORE,
    )

    t_row = nc.dram_tensor("t_row", [1, N], F32, kind="ExternalInput")
    tcol = nc.dram_tensor("tcol", [P, UT], F32, kind="ExternalInput")
    pcol = nc.dram_tensor("pcol", [P, UT], F32, kind="ExternalInput")
    pall = nc.dram_tensor("pall", [P, NUTILE], F32, kind="ExternalInput")
    iota_i = nc.dram_tensor("iota_i", [P, IS], F32, kind="ExternalInput")
    win_row = nc.dram_tensor("win_row", [1, IS], F32, kind="ExternalInput")
    out_part = nc.dram_tensor("out_part", [1, 1], F32, kind="ExternalOutput")
    if debug:
        dbg_r = nc.dram_tensor("dbg_r", [P, UT], F32, kind="ExternalOutput")
        dbg_m = nc.dram_tensor("dbg_m", [P, UT], F32, kind="ExternalOutput")
        dbg_ab = nc.dram_tensor("dbg_ab", [2, IS], F32, kind="ExternalOutput")

    with tile.TileContext(nc) as tc:
        with (
            tc.tile_pool(name="consts", bufs=1) as consts,
            tc.tile_pool(name="rep", bufs=1) as rp,
            tc.tile_pool(name="work", bufs=work_bufs) as work,
            tc.tile_pool(name="wpool", bufs=w_bufs) as wpool,
            tc.tile_pool(name="psum", bufs=1, space="PSUM") as psum,
            tc.tile_pool(name="dram", bufs=1, space="DRAM") as dram,
        ):
            # ---- constant/small loads (once) ----
            tcol_sb = consts.tile([P, UT], F32)
            nc.sync.dma_start(tcol_sb[:], tcol.ap())
            pcol_sb = consts.tile([P, UT], F32)
            nc.sync.dma_start(pcol_sb[:], pcol.ap())
            pall_sb = consts.tile([P, NUTILE], F32)
            nc.sync.dma_start(pall_sb[:], pall.ap())
            iota_sb = consts.tile([P, IS], F32)
            nc.sync.dma_start(iota_sb[:], iota_i.ap())
            win_sb = consts.tile([1, IS], F32)
            nc.sync.dma_start(win_sb[:], win_row.ap())

            negt = consts.tile([P, UT], F32)
            nc.vector.tensor_scalar(negt[:], tcol_sb[:], -1.0, None, OP.mult)
            ones_col = consts.tile([P, 1], F32)
            nc.vector.memset(ones_col[:], 1.0)

            # ab stationary: a=exp(p), b=exp(-p), interleaved [P,64,2]
            ab = consts.tile([P, NUTILE, 2], F32)
            nc.scalar.activation(ab[:, :, 0], pall_sb[:], AF.Exp)
            nc.scalar.activation(ab[:, :, 1], pall_sb[:], AF.Exp, scale=-1.0)
            if MATMUL_MODE == "bf16":
                w_dt = mybir.dt.bfloat16
            elif MATMUL_MODE == "f32r":
                w_dt = mybir.dt.float32r
            else:
                w_dt = F32
            if MATMUL_MODE == "f32":
                ab_mm = ab
            else:
                ab_mm = consts.tile([P, NUTILE, 2], w_dt)
                nc.vector.tensor_copy(ab_mm[:], ab[:])

            # phase-2 ACT offload constants: halved ab weights for the
            # {-1,+1} sign masks, plus the i-independent correction
            # K = 0.5 * sum_{u in ACT2 tiles} (a_u, b_u).
            act2 = tuple(t for t in range(NUTILE) if t % 8 in act2_mod)
            abh = consts.tile([P, NUTILE, 2], w_dt)
            nc.vector.tensor_scalar(abh[:], ab[:], 0.5, None, OP.mult)
            kpart = consts.tile([P, 2], F32)
            ab4 = ab[:].rearrange("p (g o) c -> p g o c", o=8)
            nc.vector.tensor_reduce(
                kpart[:],
                ab4[:, :, min(act2_mod) : max(act2_mod) + 1, :].transpose(
                    [0, 3, 1, 2]
                ),
                axis=mybir.AxisListType.XY,
                op=OP.add,
            )
            half_col = consts.tile([P, 1], F32)
            nc.vector.memset(half_col[:], 0.5)
            k_ps = psum.tile([2, 1], F32, tag="k_ps")
            nc.tensor.matmul(
                k_ps[:], lhsT=kpart[:], rhs=half_col[:], start=True, stop=True
            )
            k_sb = consts.tile([2, 1], F32)
            nc.vector.tensor_copy(k_sb[:], k_ps[:])

            for _rep in range(reps):
                # ---- phase 1: ranks, chunked along j for DMA overlap ----
                # separate accumulator tiles per engine so DVE and ACT never
                # share a written tile (avoids any conservative WAW hazard)
                racc_d = rp.tile([P, UT * nch], F32, tag="racc_d")
                racc_a = rp.tile([P, UT * nch], F32, tag="racc_a")
                for q in range(nch):
                    lo, hi = bounds[q], bounds[q + 1]
                    cw = hi - lo
                    tb = work.tile([P, cw], F32, tag=f"tb{q}")
                    nc.sync.dma_start(
                        tb[:],
                        t_row.ap()[0:1, lo:hi].to_broadcast((P, cw)),
                    )
                    # fixed-size scratch, sub-viewed per chunk (never read;
                    # same-engine WAW ordering is free)
                    scr_d = rp.tile([P, max(b - a for a, b in zip(bounds, bounds[1:]))],
                                    F32, tag="scr_d")
                    scr_a = rp.tile([P, max(b - a for a, b in zip(bounds, bounds[1:]))],
                                    F32, tag="scr_a")
                    for k in range(UT):
                        if k in dve_k:
                            col = racc_d[:, k * nch + q : k * nch + q + 1]
                            nc.vector.tensor_scalar(
                                scr_d[:, 0:cw], tb[:], tcol_sb[:, k : k + 1], None,
                                OP.is_gt, OP.add, accum_out=col,
                            )
                        else:
                            col = racc_a[:, k * nch + q : k * nch + q + 1]
                            nc.scalar.activation(
                                scr_a[:, 0:cw], tb[:], AF.Sign,
                                bias=negt[:, k : k + 1], accum_out=col,
                            )

                rsum = rp.tile([P, UT], F32, tag="rsum")
                nd = len(dve_k)
                nc.vector.tensor_reduce(
                    rsum[:, 0:nd],
                    racc_d[:, 0 : nd * nch].rearrange(
                        "p (k q) -> p k q", q=nch
                    ),
                    axis=mybir.AxisListType.X,
                    op=OP.add,
                )
                if act_k:
                    nc.vector.tensor_reduce(
                        rsum[:, nd:UT],
                        racc_a[:, nd * nch : UT * nch].rearrange(
                            "p (k q) -> p k q", q=nch
                        ),
                        axis=mybir.AxisListType.X,
                        op=OP.add,
                    )
                # ACT subtiles hold sum-of-sign: r = (s + N-1) / 2
                ka, kb = (min(act_k), max(act_k) + 1) if act_k else (0, 0)
                if act_k:
                    nc.vector.tensor_scalar(
                        rsum[:, ka:kb], rsum[:, ka:kb], float(N - 1), 0.5,
                        OP.add, OP.mult,
                    )
                if debug:
                    nc.sync.dma_start(dbg_r.ap(), rsum[:])

                # ---- phase 1.5: m, num partial, AllGather ----
                tmp = rp.tile([P, UT], F32, tag="tmp")
                nc.vector.tensor_scalar(
                    tmp[:], rsum[:], float(N - 1), -1.0, OP.subtract, OP.mult
                )
                m_col = rp.tile([P, UT], F32, tag="m_col")
                nc.vector.tensor_tensor(m_col[:], rsum[:], tmp[:], OP.min)
                # gather m + 0.5 so phase 2 needs no post-collective adjust:
                # [i <= m] == [i < m+0.5]; ACT sign bias uses m+0.5 directly
                nc.vector.tensor_scalar(m_col[:], m_col[:], 0.5, None, OP.add)
                if debug:
                    nc.sync.dma_start(dbg_m.ap(), m_col[:])

                sgn = rp.tile([P, UT], F32, tag="sgn")
                nc.vector.tensor_scalar(sgn[:], rsum[:], float(NPAIR), None, OP.is_lt)
                nc.vector.tensor_scalar(sgn[:], sgn[:], 2.0, -1.0, OP.mult, OP.add)
                xp = rp.tile([P, UT], F32, tag="xp")
                nc.vector.tensor_tensor(xp[:], sgn[:], pcol_sb[:], OP.mult)
                xq = rp.tile([P, 1], F32, tag="xq")
                nc.vector.tensor_reduce(
                    xq[:], xp[:], axis=mybir.AxisListType.X, op=OP.add
                )
                np_ps = psum.tile([1, 1], F32, tag="np_ps")
                nc.tensor.matmul(
                    np_ps[:], lhsT=xq[:], rhs=ones_col[:], start=True, stop=True
                )

                # partition-major m end-to-end: the pre-collective store is a
                # fully contiguous [128,8] DMA and the post-collective reload
                # reads 32B bursts instead of 4B singles.  Gathered layout is
                # [core c][partition p][subtile k]; global tile t = 8c+k, so
                # rearrange "c p k -> p (c k)" recovers mall[:, t] order.
                m_dram = dram.tile([P, UT], F32, tag="m_dram")
                nc.sync.dma_start(m_dram[:], m_col[:])
                mall_dram = dram.tile([NCORE, P, UT], F32, tag="mall_dram")
                if collective:
                    nc.gpsimd.collective_compute(
                        "AllGather",
                        OP.bypass,
                        replica_groups=[list(range(NCORE))],
                        ins=[m_dram[:].opt()],
                        outs=[mall_dram[:].opt()],
                    )
                else:  # timing-sim variant: stand-in DMA, wrong data, same shapes
                    nc.sync.dma_start(mall_dram[0, :, :], m_dram[:])
                mall = rp.tile([P, NCORE, UT], F32, tag="mall")
                mall_t = mall_dram[:].rearrange("c p k -> p c k")
                hc = NCORE // 2
                nc.sync.dma_start(mall[:, 0:hc, :], mall_t[:, 0:hc, :])
                nc.sync.dma_start(mall[:, hc:NCORE, :], mall_t[:, hc:NCORE, :])

                # ---- phase 2: A(i), B(i) via masked PE accumulation ----
                # DVE tiles: W = [i <= m_u] in {0,1}, weights ab.
                # ACT tiles: W = sign(m_u - i + 0.5) in {-1,+1}, weights ab/2,
                #            corrected by K afterwards.
                AB_ps = psum.tile([2, IS], F32, tag="AB_ps")
                for t in range(NUTILE):
                    if t % 8 in act2_mod:
                        w = wpool.tile([P, IS], w_dt, tag="wa")
                        nc.scalar.activation(
                            w[:], iota_sb[:], AF.Sign,
                            bias=mall[:, t // UT, t % UT : t % UT + 1], scale=-1.0,
                        )
                        lhsT = abh[:, t, :]
                    else:
                        w = wpool.tile([P, IS], w_dt, tag="w")
                        nc.vector.tensor_scalar(
                            w[:], iota_sb[:], mall[:, t // UT, t % UT : t % UT + 1], None, OP.is_lt
                        )
                        lhsT = ab_mm[:, t, :]
                    nc.tensor.matmul(
                        AB_ps[:], lhsT=lhsT, rhs=w[:],
                        start=(t == 0), stop=(t == NUTILE - 1),
                    )

                # ---- phase 3: loss partial ----
                AB_sb = rp.tile([2, IS], F32, tag="AB_sb")
                nc.vector.tensor_copy(AB_sb[:], AB_ps[:])
                # add the ACT2 sign-mask correction K (per-partition scalar)
                nc.vector.tensor_scalar(
                    AB_sb[:], AB_sb[:], k_sb[:], None, OP.add
                )
                if debug:
                    nc.sync.dma_start(dbg_ab.ap(), AB_sb[:])
                b_row = rp.tile([1, IS], F32, tag="b_row")
                nc.sync.dma_start(b_row[:], AB_sb[1:2, :])
                den = rp.tile([1, IS], F32, tag="den")
                nc.vector.tensor_tensor(den[:], AB_sb[0:1, :], b_row[:], OP.mult)
                nc.vector.tensor_tensor(den[:], den[:], win_sb[:], OP.subtract)
                nc.vector.tensor_scalar(den[:], den[:], 1e-8, None, OP.max)
                logd = rp.tile([1, IS], F32, tag="logd")
                logsum = rp.tile([1, 1], F32, tag="logsum")
                nc.scalar.activation(logd[:], den[:], AF.Ln, accum_out=logsum[:])
                out_sb = rp.tile([1, 1], F32, tag="out_sb")
                nc.vector.tensor_tensor(
                    out_sb[:], logsum[:], np_ps[0:1, :], OP.subtract
                )
                nc.sync.dma_start(out_part.ap(), out_sb[:])

    nc.compile()
    return nc


def make_in_maps(pred: np.ndarray, target: np.ndarray):
    pred = np.ascontiguousarray(pred, dtype=np.float32).reshape(N)
    target = np.ascontiguousarray(target, dtype=np.float32).reshape(N)
    t_row = target.reshape(1, N)
    pall = np.ascontiguousarray(pred.reshape(NUTILE, P).T)
    in_maps = []
    for c in range(NCORE):
        tsl = target[c * US : (c + 1) * US]
        psl = pred[c * US : (c + 1) * US]
        i0 = c * IS
        iv = np.arange(i0, i0 + IS, dtype=np.float32)
        in_maps.append(
            {
                "t_row": t_row,
                "tcol": np.ascontiguousarray(tsl.reshape(UT, P).T),
                "pcol": np.ascontiguousarray(psl.reshape(UT, P).T),
                "pall": pall,
                "iota_i": np.ascontiguousarray(
                    np.broadcast_to(iv, (P, IS)).astype(np.float32)
                ),
                "win_row": (N - 2.0 * iv).astype(np.float32).reshape(1, IS),
            }
        )
    return in_maps


_CACHE = {}


def _get_module():
    if "nc" not in _CACHE:
        _CACHE["nc"] = build_module(debug=False)
    return _CACHE["nc"]


def kernel(pred: np.ndarray, target: np.ndarray) -> np.ndarray:
    from concourse import bass_utils

    nc = _get_module()
    in_maps = make_in_maps(pred, target)
    res = bass_utils.run_bass_kernel_spmd(nc, in_maps, core_ids=list(range(NCORE)))
    total = np.float32(0.0)
    for c in range(NCORE):
        total = np.float32(total + res.results[c]["out_part"][0, 0])
    return np.asarray(total, dtype=np.float32)


# revision 27
# speedup vs baseline: 1.9293x; 1.9293x over previous
"""ListFoldLoss Trainium2 kernel (8-core SPMD, Bass/Tile).

Math: the reference builds D[u,v] = exp(f_u - f_v) (rank-1: exp(f) x exp(-f))
and takes nested-window sums of it.  Every window sum factorizes:

    S(i) = A(i) * B(i),   A(i) = sum_{rank in [i, N-i)} exp(pred),
                          B(i) = sum_{rank in [i, N-i)} exp(-pred)

so the NxN matrix never needs to exist.  With r_u = rank of element u when
sorting by target descending and m_u = min(r_u, N-1-r_u):

    A(i)  = sum_u exp(pred_u)  * [m_u >= i]
    B(i)  = sum_u exp(-pred_u) * [m_u >= i]
    num   = sum_u pred_u * (2*[r_u < N/2] - 1)        (= sum_i log num_i)
    loss  = sum_i log(A(i)*B(i) - (N-2i)) - num

Device plan (per core c of 8):
  phase 1: ranks r_u for u in the core's 1024-slice, via N comparisons per u:
           DVE fused tensor_scalar(is_gt)+accum_out for 5 of 8 u-subtiles and
           ACT fused sign(t_j - t_u)+accum_out for 3 (runs concurrently).
  phase 1.5: m = min(r, N-1-r), local num partial; AllGather m (4KB/core).
  phase 2: W[u,i] = [i <= m_u] (DVE tensor_scalar is_le, 2x mode) and
           PSUM accumulation of (a,b)^T @ W over 64 u-subtiles (PE matmuls).
  phase 3: denom = A*B - win, ln via ACT (+fused sum), minus num partial.
Host: sums the 8 per-core partial losses (pure unshard of a sum-sharded scalar).

`reps` replicates the whole body serially inside one NEFF — used only for
slope-based wall-clock timing (NTFF profiling is unavailable here).
"""

import numpy as np

import concourse.bacc as bacc
import concourse.bass as bass
import concourse.mybir as mybir
import concourse.tile as tile

N = 8192
NCORE = 8
P = 128
US = N // NCORE          # 1024 u's per core
UT = US // P             # 8 u-subtiles per core
NPAIR = N // 2           # 4096 loss terms
IS = NPAIR // NCORE      # 512 i's per core
NCHUNK = 2               # j-dim chunks for DMA/compute pipelining
CH = N // NCHUNK
NUTILE = N // P          # 64 u-subtiles globally (phase 2)
DVE_K = (0, 1, 2, 3, 4)  # u-subtiles ranked on the vector engine
ACT_K = (5, 6, 7)        # u-subtiles ranked on the scalar engine (sign trick)
# phase-2 mask tiles handled by ACT (sign trick, mod-16 position), rest on DVE
# 20/64 on ACT balances ACT=39.5us vs DVE=39.0us busy (mod-8's 24 over-loads ACT)
ACT2_MOD = (5, 6, 7, 13, 14)

F32 = mybir.dt.float32
AF = mybir.ActivationFunctionType
OP = mybir.AluOpType

# "f32" = correct but 4 cyc/row on PE; "f32r" = full rate, reduced precision;
# "bf16" = full rate, ab operands rounded to bf16.
MATMUL_MODE = "f32r"


def build_module(
    debug: bool = False,
    reps: int = 1,
    collective: bool = True,
    n1d: int = len(DVE_K),
    act2_mod: tuple = ACT2_MOD,
    nchunk: int = NCHUNK,
    work_bufs: int = 2,
    w_bufs: int = 4,
):
    dve_k = tuple(range(n1d))
    act_k = tuple(range(n1d, UT))
    # progressive chunk sizes: small first chunk so compares start early,
    # large later chunks to amortize per-instruction overhead
    if nchunk == 2:
        bounds = (0, 1024, 4096, N)
    else:
        bounds = tuple(q * (N // nchunk) for q in range(nchunk)) + (N,)
    nch = len(bounds) - 1
    nc = bacc.Bacc(
        "TRN2",
        target_bir_lowering=False,
        debug=False,
        enable_asserts=False,
        num_devices=NCORE,
    )

    t_row = nc.dram_tensor("t_row", [1, N], F32, kind="ExternalInput")
    # packed small consts: [tcol 8 | pcol 8 | pall 64 | win_pm 4] per partition
    NPK = UT + UT + NUTILE + IS // P
    packed = nc.dram_tensor("packed", [P, NPK], F32, kind="ExternalInput")
    iota_i = nc.dram_tensor("iota_i", [P, IS], F32, kind="ExternalInput")
    eye2 = nc.dram_tensor("eye2", [2, 2], F32, kind="ExternalInput")
    out_part = nc.dram_tensor("out_part", [1, 1], F32, kind="ExternalOutput")
    if debug:
        dbg_r = nc.dram_tensor("dbg_r", [P, UT], F32, kind="ExternalOutput")
        dbg_m = nc.dram_tensor("dbg_m", [P, UT], F32, kind="ExternalOutput")
        dbg_ab = nc.dram_tensor("dbg_ab", [2, IS], F32, kind="ExternalOutput")

    with tile.TileContext(nc) as tc:
        with (
            tc.tile_pool(name="consts", bufs=1) as consts,
            tc.tile_pool(name="rep", bufs=1) as rp,
            tc.tile_pool(name="work", bufs=work_bufs) as work,
            tc.tile_pool(name="wpool", bufs=w_bufs) as wpool,
            tc.tile_pool(name="psum", bufs=1, space="PSUM") as psum,
            tc.tile_pool(name="dram", bufs=1, space="DRAM") as dram,
        ):
            # ---- constant/small loads (once, one packed DMA) ----
            # packed rides the scalar-engine ring so tb0 leads the sync ring:
            # the first compare needs BOTH; parallel FIFOs overlap their fixed costs
            packed_sb = consts.tile([P, NPK], F32)
            nc.scalar.dma_start(packed_sb[:], packed.ap())
            tcol_sb = packed_sb[:, 0:UT]
            pcol_sb = packed_sb[:, UT : 2 * UT]
            pall_sb = packed_sb[:, 2 * UT : 2 * UT + NUTILE]
            win_sb = packed_sb[:, 2 * UT + NUTILE : NPK]
            # iota/eye2 ride the scalar-engine HWDGE ring so they don't
            # queue ahead of the first compare chunk on the sync-engine FIFO
            iota_sb = consts.tile([P, IS], F32)
            nc.scalar.dma_start(iota_sb[:], iota_i.ap())
            eye2_sb = consts.tile([2, 2], F32)
            nc.scalar.dma_start(eye2_sb[:], eye2.ap())

            negt = consts.tile([P, UT], F32)
            nc.vector.tensor_scalar(negt[:], tcol_sb, -1.0, None, OP.mult)
            ones_col = consts.tile([P, 1], F32)
            nc.vector.memset(ones_col[:], 1.0)

            # ab stationary: a=exp(p), b=exp(-p), interleaved [P,64,2]
            ab = consts.tile([P, NUTILE, 2], F32)
            nc.scalar.activation(ab[:, :, 0], pall_sb, AF.Exp)
            nc.scalar.activation(ab[:, :, 1], pall_sb, AF.Exp, scale=-1.0)
            if MATMUL_MODE == "bf16":
                w_dt = mybir.dt.bfloat16
            elif MATMUL_MODE == "f32r":
                w_dt = mybir.dt.float32r
            else:
                w_dt = F32
            if MATMUL_MODE == "f32":
                ab_mm = ab
            else:
                ab_mm = consts.tile([P, NUTILE, 2], w_dt)
                nc.vector.tensor_copy(ab_mm[:], ab[:])

            # phase-2 ACT offload constants: halved ab weights for the
            # {-1,+1} sign masks, plus the i-independent correction
            # K = 0.5 * sum_{u in ACT2 tiles} (a_u, b_u).
            abh = consts.tile([P, NUTILE, 2], w_dt)
            nc.vector.tensor_scalar(abh[:], ab[:], 0.5, None, OP.mult)
            # K = sum of ab over the ACT2 tile set; the mod-16 set is two
            # contiguous runs, reduced separately and added
            runs = []
            run = []
            for o in sorted(act2_mod):
                if run and o != run[-1] + 1:
                    runs.append(run)
                    run = []
                run.append(o)
            runs.append(run)
            ab16 = ab[:].rearrange("p (g o) c -> p g o c", o=16)
            kparts = []
            for run in runs:
                kp = consts.tile([P, 2], F32, tag=f"kp{run[0]}")
                nc.vector.tensor_reduce(
                    kp[:],
                    ab16[:, :, run[0] : run[-1] + 1, :].transpose([0, 3, 1, 2]),
                    axis=mybir.AxisListType.XY,
                    op=OP.add,
                )
                kparts.append(kp)
            kpart = consts.tile([P, 2], F32)
            if len(kparts) == 1:
                kpart = kparts[0]
            else:
                nc.vector.tensor_tensor(
                    kpart[:], kparts[0][:], kparts[1][:], OP.add
                )
            half_col = consts.tile([P, 1], F32)
            nc.vector.memset(half_col[:], 0.5)
            k_ps = psum.tile([2, 1], F32, tag="k_ps")
            nc.tensor.matmul(
                k_ps[:], lhsT=kpart[:], rhs=half_col[:], start=True, stop=True
            )
            k_sb = consts.tile([2, 1], F32)
            nc.vector.tensor_copy(k_sb[:], k_ps[:])

            for _rep in range(reps):
                # ---- phase 1: ranks, chunked along j for DMA overlap ----
                # separate accumulator tiles per engine so DVE and ACT never
                # share a written tile (avoids any conservative WAW hazard)
                racc_d = rp.tile([P, UT * nch], F32, tag="racc_d")
                racc_a = rp.tile([P, UT * nch], F32, tag="racc_a")
                for q in range(nch):
                    lo, hi = bounds[q], bounds[q + 1]
                    cw = hi - lo
                    tb = work.tile([P, cw], F32, tag=f"tb{q}")
                    nc.sync.dma_start(
                        tb[:],
                        t_row.ap()[0:1, lo:hi].to_broadcast((P, cw)),
                    )
                    # fixed-size scratch, sub-viewed per chunk (never read;
                    # same-engine WAW ordering is free)
                    scr_d = rp.tile([P, max(b - a for a, b in zip(bounds, bounds[1:]))],
                                    F32, tag="scr_d")
                    scr_a = rp.tile([P, max(b - a for a, b in zip(bounds, bounds[1:]))],
                                    F32, tag="scr_a")
                    for k in range(UT):
                        if k in dve_k:
                            col = racc_d[:, k * nch + q : k * nch + q + 1]
                            nc.vector.tensor_scalar(
                                scr_d[:, 0:cw], tb[:], tcol_sb[:, k : k + 1], None,
                                OP.is_gt, OP.add, accum_out=col,
                            )
                        else:
                            col = racc_a[:, k * nch + q : k * nch + q + 1]
                            nc.scalar.activation(
                                scr_a[:, 0:cw], tb[:], AF.Sign,
                                bias=negt[:, k : k + 1], accum_out=col,
                            )

                rsum = rp.tile([P, UT], F32, tag="rsum")
                nd = len(dve_k)
                nc.vector.tensor_reduce(
                    rsum[:, 0:nd],
                    racc_d[:, 0 : nd * nch].rearrange(
                        "p (k q) -> p k q", q=nch
                    ),
                    axis=mybir.AxisListType.X,
                    op=OP.add,
                )
                if act_k:
                    nc.vector.tensor_reduce(
                        rsum[:, nd:UT],
                        racc_a[:, nd * nch : UT * nch].rearrange(
                            "p (k q) -> p k q", q=nch
                        ),
                        axis=mybir.AxisListType.X,
                        op=OP.add,
                    )
                # ACT subtiles hold sum-of-sign: r = (s + N-1) / 2
                ka, kb = (min(act_k), max(act_k) + 1) if act_k else (0, 0)
                if act_k:
                    nc.vector.tensor_scalar(
                        rsum[:, ka:kb], rsum[:, ka:kb], float(N - 1), 0.5,
                        OP.add, OP.mult,
                    )
                if debug:
                    nc.sync.dma_start(dbg_r.ap(), rsum[:])

                # ---- phase 1.5: m, num partial, AllGather ----
                tmp = rp.tile([P, UT], F32, tag="tmp")
                nc.vector.tensor_scalar(
                    tmp[:], rsum[:], float(N - 1), -1.0, OP.subtract, OP.mult
                )
                m_col = rp.tile([P, UT], F32, tag="m_col")
                nc.vector.tensor_tensor(m_col[:], rsum[:], tmp[:], OP.min)
                # gather m + 0.5 so phase 2 needs no post-collective adjust:
                # [i <= m] == [i < m+0.5]; ACT sign bias uses m+0.5 directly
                nc.vector.tensor_scalar(m_col[:], m_col[:], 0.5, None, OP.add)
                if debug:
                    nc.sync.dma_start(dbg_m.ap(), m_col[:])

                sgn = rp.tile([P, UT], F32, tag="sgn")
                nc.vector.tensor_scalar(sgn[:], rsum[:], float(NPAIR), None, OP.is_lt)
                nc.vector.tensor_scalar(sgn[:], sgn[:], 2.0, -1.0, OP.mult, OP.add)
                xp = rp.tile([P, UT], F32, tag="xp")
                nc.vector.tensor_tensor(xp[:], sgn[:], pcol_sb, OP.mult)
                xq = rp.tile([P, 1], F32, tag="xq")
                nc.vector.tensor_reduce(
                    xq[:], xp[:], axis=mybir.AxisListType.X, op=OP.add
                )
                np_ps = psum.tile([1, 1], F32, tag="np_ps")
                nc.tensor.matmul(
                    np_ps[:], lhsT=xq[:], rhs=ones_col[:], start=True, stop=True
                )

                # partition-major m end-to-end: the pre-collective store is a
                # fully contiguous [128,8] DMA and the post-collective reload
                # reads 32B bursts instead of 4B singles.  Gathered layout is
                # [core c][partition p][subtile k]; global tile t = 8c+k, so
                # rearrange "c p k -> p (c k)" recovers mall[:, t] order.
                m_dram = dram.tile([P, UT], F32, tag="m_dram")
                nc.sync.dma_start(m_dram[:], m_col[:])
                mall_dram = dram.tile([NCORE, P, UT], F32, tag="mall_dram")
                if collective:
                    nc.gpsimd.collective_compute(
                        "AllGather",
                        OP.bypass,
                        replica_groups=[list(range(NCORE))],
                        ins=[m_dram[:].opt()],
                        outs=[mall_dram[:].opt()],
                    )
                else:  # timing-sim variant: stand-in DMA, wrong data, same shapes
                    nc.sync.dma_start(mall_dram[0, :, :], m_dram[:])
                mall = rp.tile([P, NCORE, UT], F32, tag="mall")
                mall_t = mall_dram[:].rearrange("c p k -> p c k")
                hc = NCORE // 2
                nc.sync.dma_start(mall[:, 0:hc, :], mall_t[:, 0:hc, :])
                nc.sync.dma_start(mall[:, hc:NCORE, :], mall_t[:, hc:NCORE, :])

                # ---- phase 2: A(i), B(i) via masked PE accumulation ----
                # DVE tiles: W = [i <= m_u] in {0,1}, weights ab.
                # ACT tiles: W = sign(m_u - i + 0.5) in {-1,+1}, weights ab/2,
                #            corrected by K afterwards.
                AB_ps = psum.tile([2, IS], F32, tag="AB_ps")
                for t in range(NUTILE):
                    if t % 16 in act2_mod:
                        w = wpool.tile([P, IS], w_dt, tag="wa")
                        nc.scalar.activation(
                            w[:], iota_sb[:], AF.Sign,
                            bias=mall[:, t // UT, t % UT : t % UT + 1], scale=-1.0,
                        )
                        lhsT = abh[:, t, :]
                    else:
                        w = wpool.tile([P, IS], w_dt, tag="w")
                        nc.vector.tensor_scalar(
                            w[:], iota_sb[:], mall[:, t // UT, t % UT : t % UT + 1], None, OP.is_lt
                        )
                        lhsT = ab_mm[:, t, :]
                    nc.tensor.matmul(
                        AB_ps[:], lhsT=lhsT, rhs=w[:],
                        start=(t == 0), stop=(t == NUTILE - 1),
                    )

                # ---- phase 3: loss partial, partition-parallel ----
                # PE-transpose [2, 512] -> 4x [128, 2] so denom/ln run on all
                # 128 lanes and no cross-partition DMA sits on the tail.
                AB_sb = rp.tile([2, IS], F32, tag="AB_sb")
                nc.vector.tensor_copy(AB_sb[:], AB_ps[:])
                # add the ACT2 sign-mask correction K (per-partition scalar)
                nc.vector.tensor_scalar(
                    AB_sb[:], AB_sb[:], k_sb[:], None, OP.add
                )
                if debug:
                    nc.sync.dma_start(dbg_ab.ap(), AB_sb[:])
                ng = IS // P
                tr_ps = psum.tile([P, ng, 2], F32, tag="tr_ps")
                for g in range(ng):
                    nc.tensor.transpose(
                        tr_ps[:, g, :], AB_sb[:, g * P : (g + 1) * P], eye2_sb[:]
                    )
                tr_sb = rp.tile([P, ng, 2], F32, tag="tr_sb")
                nc.vector.tensor_copy(tr_sb[:], tr_ps[:])
                den = rp.tile([P, ng], F32, tag="den")
                nc.vector.tensor_tensor(
                    den[:], tr_sb[:, :, 0], tr_sb[:, :, 1], OP.mult
                )
                nc.vector.tensor_tensor(den[:], den[:], win_sb, OP.subtract)
                nc.vector.tensor_scalar(den[:], den[:], 1e-8, None, OP.max)
                logd = rp.tile([P, ng], F32, tag="logd")
                lnacc = rp.tile([P, 1], F32, tag="lnacc")
                nc.scalar.activation(logd[:], den[:], AF.Ln, accum_out=lnacc[:])
                ln_ps = psum.tile([1, 1], F32, tag="ln_ps")
                nc.tensor.matmul(
                    ln_ps[:], lhsT=lnacc[:], rhs=ones_col[:], start=True, stop=True
                )
                ln_sb = rp.tile([1, 1], F32, tag="ln_sb")
                nc.vector.tensor_copy(ln_sb[:], ln_ps[:])
                out_sb = rp.tile([1, 1], F32, tag="out_sb")
                nc.vector.tensor_tensor(
                    out_sb[:], ln_sb[:], np_ps[0:1, :], OP.subtract
                )
                nc.sync.dma_start(out_part.ap(), out_sb[:])

    nc.compile()
    return nc


def make_in_maps(pred: np.ndarray, target: np.ndarray):
    pred = np.ascontiguousarray(pred, dtype=np.float32).reshape(N)
    target = np.ascontiguousarray(target, dtype=np.float32).reshape(N)
    t_row = target.reshape(1, N)
    pall = np.ascontiguousarray(pred.reshape(NUTILE, P).T)
    in_maps = []
    for c in range(NCORE):
        tsl = target[c * US : (c + 1) * US]
        psl = pred[c * US : (c + 1) * US]
        i0 = c * IS
        iv = np.arange(i0, i0 + IS, dtype=np.float32)
        pk = np.concatenate(
            [
                tsl.reshape(UT, P).T,
                psl.reshape(UT, P).T,
                pall,
                (N - 2.0 * iv).astype(np.float32).reshape(IS // P, P).T,
            ],
            axis=1,
        ).astype(np.float32)
        in_maps.append(
            {
                "t_row": t_row,
                "packed": np.ascontiguousarray(pk),
                "iota_i": np.ascontiguousarray(
                    np.broadcast_to(iv, (P, IS)).astype(np.float32)
                ),
                "eye2": np.eye(2, dtype=np.float32),
            }
        )
    return in_maps


_CACHE = {}


def _get_module():
    if "nc" not in _CACHE:
        _CACHE["nc"] = build_module(debug=False)
    return _CACHE["nc"]


def kernel(pred: np.ndarray, target: np.ndarray) -> np.ndarray:
    from concourse import bass_utils

    nc = _get_module()
    in_maps = make_in_maps(pred, target)
    res = bass_utils.run_bass_kernel_spmd(nc, in_maps, core_ids=list(range(NCORE)))
    total = np.float32(0.0)
    for c in range(NCORE):
        total = np.float32(total + res.results[c]["out_part"][0, 0])
    return np.asarray(total, dtype=np.float32)
